# revision 1
# baseline (speedup 1.0000x reference)
"""Trainium2 Bass kernel for nn_Critic (RnnEncoder + attention critic).

Data-parallel over batch B=256 across 8 cores (32 batches/core).
Layout is channel-transposed: channels on partitions, batch rows on the free
dim.  Per core the encoder GRU runs 256 rows (=32 b x 8 d) and the decoder
GRU 32 rows, folded as columns 256:288 of the same elementwise stream.

GRU step (torch convention):
  pre_rz = Whh_rz @ h + Wih_rz @ x + (bih+bhh)_rz      (PE, ones-row bias)
  r, z = sigmoid(pre_rz)                                (ACT, bias=0)
  hn = Whh_n @ h          (PE)   inn = Wih_n @ x + bih_n (PE, ones row)
  tmp = (hn + bhh_n[p]) * r                             (DVE STT)
  npre = tmp + inn                                      (DVE TT)
  n = tanh(npre)                                        (ACT)
  d = h - n (Pool)   zd = z * d (DVE)   h' = n + zd     (DVE)
"""

import numpy as np
import ml_dtypes

import concourse.bass as bass
import concourse.mybir as mybir
from concourse.tile import TileContext
from concourse.bass_utils import run_bass_kernel_spmd

B, N, D, H = 256, 128, 8, 256
NCORES = 8
BC = B // NCORES        # 32 batches per core
RE = BC * D             # 256 encoder rows per core
RD = BC                 # 32 decoder rows per core
NCOL = RE + RD          # 288 columns in the folded elementwise stream
T = N                   # 128 time steps
H2 = 2 * H              # 512 (r,z channels)

f32 = mybir.dt.float32
bf16 = mybir.dt.bfloat16
AF = mybir.ActivationFunctionType
OP = mybir.AluOpType
NPBF = ml_dtypes.bfloat16

TRACE = False
LAST_RESULT = [None]

_BUILT = {}


def _split_excess_waits(nc, max_waits=1):
    """This walrus build encodes at most one sync-wait per instruction for
    several instruction classes (CTRL/DMA).  Hoist extras onto nops."""
    for f in nc.m.functions:
        for bb in f.blocks:
            out = []
            for ins in bb.instructions:
                si = getattr(ins, "sync_info", None)
                if si is not None and len(si.on_wait) > max_waits:
                    waits = list(si.on_wait)
                    keep, extra = waits[-max_waits:], waits[:-max_waits]
                    for w in extra:
                        nop = mybir.InstNoOp(
                            name=nc.get_next_instruction_name(), ins=[], outs=[])
                        nop.engine = ins.engine
                        nop.sync_info = mybir.SyncInfo(on_wait=[w], on_update=[])
                        nc.register_instruction(nop, overwrite=True)
                        out.append(nop)
                    si.on_wait.clear()
                    for w in keep:
                        si.on_wait.append(w)
                out.append(ins)
            bb.instructions[:] = out


def _build(reps=1, mode="full"):
    """reps>1 wraps the GRU time loop in a device-side repeat loop (timing
    harness only -- output is then garbage but numerically bounded).
    mode: "full" | "pe_only" (GRU matmuls only) | "nodec" (encoder only)."""
    nc = bass.Bass(trn_type="TRN2")

    def din(name, shape, dt):
        return nc.dram_tensor(name, shape, dt, kind="ExternalInput")

    xe_d = din("xe", [3, T, RE], bf16)
    xd_d = din("xd", [3, T, RD], bf16)
    whh_rz_e_d = din("whh_rz_e", [128, 2, H2], bf16)
    whh_n_e_d = din("whh_n_e", [128, 2, H], bf16)
    whh_rz_d_d = din("whh_rz_d", [128, 2, H2], bf16)
    whh_n_d_d = din("whh_n_d", [128, 2, H], bf16)
    wihb_rz_e_d = din("wihb_rz_e", [128, H2], bf16)
    wihb_n_e_d = din("wihb_n_e", [128, H], bf16)
    wihb_rz_d_d = din("wihb_rz_d", [128, H2], bf16)
    wihb_n_d_d = din("wihb_n_d", [128, H], bf16)
    b_hn_e_d = din("b_hn_e", [128, 2], f32)
    b_hn_d_d = din("b_hn_d", [128, 2], f32)
    wih_g_d = din("wih_g", [128, 2, 3 * H], bf16)
    b_rz_g_pos_d = din("b_rz_g_pos", [128, 2], f32)
    b_rz_g_neg_d = din("b_rz_g_neg", [128, 2], f32)
    b_hn_g_d = din("b_hn_g", [128, 2], f32)
    b_in_g_d = din("b_in_g", [128, 2], f32)
    a1t_d = din("a1t", [128, 2, H], bf16)
    a2t_d = din("a2t", [128, 2, H], bf16)
    attv_d = din("attv", [128, 2], bf16)
    fc1t_d = din("fc1t", [128, 2, H], bf16)
    b_fc1_d = din("b_fc1", [128, 2], f32)
    fc2t_d = din("fc2t", [128, 2], bf16)
    b_fc2_d = din("b_fc2", [1, 1], f32)

    y_d = nc.dram_tensor("y", [1, RD], f32, kind="ExternalOutput")

    EC = slice(0, RE)      # encoder columns
    DC = slice(RE, NCOL)   # decoder columns

    with TileContext(nc) as tc:
        with tc.tile_pool(name="const", bufs=1) as cp, \
             tc.tile_pool(name="state", bufs=1) as hp, \
             tc.tile_pool(name="work", bufs=2) as wp:

            def load(dram, shape, dt, name):
                t_ = cp.tile(shape, dt, name=name)
                nc.sync.dma_start(out=t_[:], in_=dram[:])
                return t_

            # x operands zero-padded to K=128 so every PE matmul is a
            # uniform K=128 instruction (K-regime switches measured ~+100ns/MM)
            xe = cp.tile([128, T, RE], bf16, name="xe_s")
            xd = cp.tile([128, T, RD], bf16, name="xd_s")
            nc.vector.memset(xe[:], 0.0)
            nc.vector.memset(xd[:], 0.0)
            nc.sync.dma_start(out=xe[0:3, :, :], in_=xe_d[:])
            nc.sync.dma_start(out=xd[0:3, :, :], in_=xd_d[:])
            whh_rz_e = load(whh_rz_e_d, [128, 2, H2], bf16, "whhrze_s")
            whh_n_e = load(whh_n_e_d, [128, 2, H], bf16, "whhne_s")
            whh_rz_d = load(whh_rz_d_d, [128, 2, H2], bf16, "whhrzd_s")
            whh_n_d = load(whh_n_d_d, [128, 2, H], bf16, "whhnd_s")
            wihb_rz_e = load(wihb_rz_e_d, [128, H2], bf16, "wihbrze_s")
            wihb_n_e = load(wihb_n_e_d, [128, H], bf16, "wihbne_s")
            wihb_rz_d = load(wihb_rz_d_d, [128, H2], bf16, "wihbrzd_s")
            wihb_n_d = load(wihb_n_d_d, [128, H], bf16, "wihbnd_s")
            b_hn_e = load(b_hn_e_d, [128, 2], f32, "bhne_s")
            b_hn_d = load(b_hn_d_d, [128, 2], f32, "bhnd_s")
            wih_g = load(wih_g_d, [128, 2, 3 * H], bf16, "wihg_s")
            b_rz_g_pos = load(b_rz_g_pos_d, [128, 2], f32, "brzgp_s")
            b_rz_g_neg = load(b_rz_g_neg_d, [128, 2], f32, "brzgn_s")
            b_hn_g = load(b_hn_g_d, [128, 2], f32, "bhng_s")
            b_in_g = load(b_in_g_d, [128, 2], f32, "bing_s")
            a1t = load(a1t_d, [128, 2, H], bf16, "a1t_s")
            a2t = load(a2t_d, [128, 2, H], bf16, "a2t_s")
            attv = load(attv_d, [128, 2], bf16, "attv_s")
            fc1t = load(fc1t_d, [128, 2, H], bf16, "fc1t_s")
            b_fc1 = load(b_fc1_d, [128, 2], f32, "bfc1_s")
            fc2t = load(fc2t_d, [128, 2], bf16, "fc2t_s")
            b_fc2 = load(b_fc2_d, [1, 1], f32, "bfc2_s")

            # persistent hidden state, channel-transposed [H, 288], bf16
            h_sb = [hp.tile([128, NCOL], bf16, name=f"h{k}") for k in range(2)]
            nc.vector.memset(h_sb[0][:], 0.0)
            nc.vector.memset(h_sb[1][:], 0.0)

            ones1 = hp.tile([1, 128], bf16, name="ones1")
            nc.vector.memset(ones1[:], 1.0)

            # ---------------- GRU time loop ----------------
            import contextlib
            rep_ctx = (tc.For_i(0, reps, 1) if reps > 1
                       else contextlib.nullcontext())
            with tc.tile_pool(name="gpsum", bufs=1, space="PSUM") as gp, rep_ctx:
                for t in range(T):
                    if mode.startswith("mm"):
                        # PE microbench: 12 back-to-back MMs per "step"
                        nn = 512 if "512" in mode else 256
                        if "acc2" in mode:
                            # same-bank accum MMs spaced 1 apart (A0 B0 A1 B1 A2 B2)
                            ps = [gp.tile([128, nn], f32, name=f"mmb{j}")
                                  for j in range(4)]
                            for rep2 in range(2):
                                for j in range(6):
                                    bank = 2 * rep2 + (j % 2)
                                    acc = j // 2
                                    m = (2 * rep2 + j) % 4
                                    ms = slice(m * 128, (m + 1) * 128)
                                    nc.tensor.matmul(
                                        out=ps[bank][:],
                                        lhsT=whh_rz_e[:, 1, ms],
                                        rhs=whh_rz_e[:, 0, 0:nn],
                                        start=(acc == 0), stop=(acc == 2))
                        elif "k3p" in mode:
                            # K=3 MMs paired at the end of each 6-MM block
                            ps = [gp.tile([128, nn], f32, name=f"mmb{j}")
                                  for j in range(2)]
                            for rep2 in range(2):
                                for j in range(4):
                                    ms = slice((j % 4) * 128, (j % 4 + 1) * 128)
                                    nc.tensor.matmul(
                                        out=ps[j % 2][:],
                                        lhsT=whh_rz_e[:, 1, ms],
                                        rhs=whh_rz_e[:, 0, 0:nn],
                                        start=True, stop=True)
                                for j in range(2):
                                    ms = slice(j * 128, (j + 1) * 128)
                                    nc.tensor.matmul(
                                        out=ps[j % 2][:], lhsT=wihb_rz_e[:, ms],
                                        rhs=xe[0:3, t, 0:nn],
                                        start=True, stop=True)
                        elif "acc" in mode:
                            ps = [gp.tile([128, nn], f32, name=f"mmb{j}")
                                  for j in range(4)]
                            for j in range(12):
                                m = j % 4
                                ms = slice(m * 128, (m + 1) * 128)
                                nc.tensor.matmul(
                                    out=ps[j // 3][:],
                                    lhsT=whh_rz_e[:, 1, ms],
                                    rhs=whh_rz_e[:, 0, 0:nn],
                                    start=(j % 3 == 0), stop=(j % 3 == 2))
                        elif "k3" in mode:
                            ps = [gp.tile([128, nn], f32, name=f"mmb{j}")
                                  for j in range(2)]
                            for j in range(12):
                                m = j % 4
                                ms = slice(m * 128, (m + 1) * 128)
                                if j % 3 == 2:
                                    nc.tensor.matmul(
                                        out=ps[j % 2][:], lhsT=wihb_rz_e[:, ms],
                                        rhs=xe[0:3, t, 0:nn],
                                        start=True, stop=True)
                                else:
                                    nc.tensor.matmul(
                                        out=ps[j % 2][:],
                                        lhsT=whh_rz_e[:, 1, ms],
                                        rhs=whh_rz_e[:, 0, 0:nn],
                                        start=True, stop=True)
                        else:
                            ps = [gp.tile([128, nn], f32, name=f"mmb{j}")
                                  for j in range(2)]
                            for j in range(12):
                                m = (j % 4) if "same" not in mode else 0
                                ms = slice(m * 128, (m + 1) * 128)
                                nc.tensor.matmul(
                                    out=ps[j % 2][:],
                                    lhsT=whh_rz_e[:, 1, ms],
                                    rhs=whh_rz_e[:, 0, 0:nn],
                                    start=True, stop=True)
                        continue
                    xer = xe[:, t, :]            # [128, RE] (rows 3+ are zero)
                    xdr = xd[:, t, :]            # [128, RD]

                    rz_ps = [gp.tile([128, NCOL], f32, name=f"rz{m}")
                             for m in range(4)]
                    hn_ps = [gp.tile([128, NCOL], f32, name=f"hnp{k}")
                             for k in range(2)]
                    in_ps = [gp.tile([128, NCOL], f32, name=f"inp{k}")
                             for k in range(2)]

                    DEC = mode not in ("nodec", "pe_nodec")

                    def mm(o, w, r_, st, sp):
                        nc.tensor.matmul(out=o, lhsT=w, rhs=r_, start=st, stop=sp)

                    def MS(m):
                        return slice(m * 128, (m + 1) * 128)

                    def rz_block(ma, mb):
                        # bank-interleaved, uniform K=128, k1 parts last
                        for m in (ma, mb):
                            mm(rz_ps[m][:, EC], whh_rz_e[:, 0, MS(m)],
                               h_sb[0][:, EC], True, False)
                        if DEC:
                            for m in (ma, mb):
                                mm(rz_ps[m][:, DC], whh_rz_d[:, 0, MS(m)],
                                   h_sb[0][:, DC], False, False)
                        for m in (ma, mb):
                            mm(rz_ps[m][:, EC], wihb_rz_e[:, MS(m)],
                               xer, False, False)
                        if DEC:
                            for m in (ma, mb):
                                mm(rz_ps[m][:, DC], wihb_rz_d[:, MS(m)],
                                   xdr, False, False)
                        for m in (ma, mb):
                            mm(rz_ps[m][:, EC], whh_rz_e[:, 1, MS(m)],
                               h_sb[1][:, EC], False, not DEC)
                        if DEC:
                            for m in (ma, mb):
                                mm(rz_ps[m][:, DC], whh_rz_d[:, 1, MS(m)],
                                   h_sb[1][:, DC], False, True)

                    def hn_inn_block():
                        for k in (0, 1):
                            mm(hn_ps[k][:, EC], whh_n_e[:, 0, MS(k)],
                               h_sb[0][:, EC], True, False)
                        if DEC:
                            for k in (0, 1):
                                mm(hn_ps[k][:, DC], whh_n_d[:, 0, MS(k)],
                                   h_sb[0][:, DC], False, False)
                        for k in (0, 1):
                            mm(in_ps[k][:, EC], wihb_n_e[:, MS(k)],
                               xer, True, not DEC)
                        if DEC:
                            for k in (0, 1):
                                mm(in_ps[k][:, DC], wihb_n_d[:, MS(k)],
                                   xdr, False, True)
                        for k in (0, 1):
                            mm(hn_ps[k][:, EC], whh_n_e[:, 1, MS(k)],
                               h_sb[1][:, EC], False, not DEC)
                        if DEC:
                            for k in (0, 1):
                                mm(hn_ps[k][:, DC], whh_n_d[:, 1, MS(k)],
                                   h_sb[1][:, DC], False, True)

                    rz_block(0, 1)
                    hn_inn_block()
                    rz_block(2, 3)

                    if mode in ("pe_only", "pe_nodec"):
                        continue

                    r_sb = [wp.tile([128, NCOL], bf16, name=f"r{k}")
                            for k in range(2)]
                    z_sb = [wp.tile([128, NCOL], bf16, name=f"z{k}")
                            for k in range(2)]
                    n_sb = [wp.tile([128, NCOL], bf16, name=f"n{k}")
                            for k in range(2)]
                    tmp_sb = [wp.tile([128, NCOL], bf16, name=f"tmp{k}")
                              for k in range(2)]
                    npre_sb = [wp.tile([128, NCOL], bf16, name=f"npre{k}")
                               for k in range(2)]
                    zc_sb = [wp.tile([128, NCOL], bf16, name=f"zc{k}")
                             for k in range(2)]
                    b_sb = [wp.tile([128, NCOL], bf16, name=f"b{k}")
                            for k in range(2)]
                    a_sb = [wp.tile([128, NCOL], bf16, name=f"a{k}")
                            for k in range(2)]

                    for k in range(2):
                        nc.scalar.activation(out=r_sb[k][:], in_=rz_ps[k][:],
                                             func=AF.Sigmoid)
                    for k in range(2):
                        nc.scalar.activation(out=z_sb[k][:], in_=rz_ps[2 + k][:],
                                             func=AF.Sigmoid)
                    for k in range(2):
                        # off-chain b = z*h_old on Pool
                        nc.gpsimd.tensor_mul(out=b_sb[k][:], in0=z_sb[k][:],
                                             in1=h_sb[k][:])
                    for k in range(2):
                        # chain: tmp = (hn + bhh_n) * r (split enc/dec biases)
                        nc.vector.scalar_tensor_tensor(
                            out=tmp_sb[k][:, EC], in0=hn_ps[k][:, EC],
                            scalar=b_hn_e[:, k:k + 1], in1=r_sb[k][:, EC],
                            op0=OP.add, op1=OP.mult)
                        if DEC:
                            nc.vector.scalar_tensor_tensor(
                                out=tmp_sb[k][:, DC], in0=hn_ps[k][:, DC],
                                scalar=b_hn_d[:, k:k + 1], in1=r_sb[k][:, DC],
                                op0=OP.add, op1=OP.mult)
                        nc.vector.tensor_add(out=npre_sb[k][:],
                                             in0=tmp_sb[k][:], in1=in_ps[k][:])
                        nc.scalar.activation(out=n_sb[k][:], in_=npre_sb[k][:],
                                             func=AF.Tanh)
                        # zc = 1-z (off-chain, emitted late so it never
                        # outranks chain ops in the scheduler heap)
                        nc.vector.tensor_scalar(
                            out=zc_sb[k][:], in0=z_sb[k][:], scalar1=-1.0,
                            scalar2=1.0, op0=OP.mult, op1=OP.add)
                        # h' = zc*n + z*h_old
                        nc.vector.tensor_mul(out=a_sb[k][:], in0=zc_sb[k][:],
                                             in1=n_sb[k][:])
                        nc.vector.tensor_add(out=h_sb[k][:], in0=a_sb[k][:],
                                             in1=b_sb[k][:])

            # ---------------- critic GRU (single step, h0=0) ----------------
            rnn = [wp.tile([128, RD], bf16, name=f"rnn{k}") for k in range(2)]
            with tc.tile_pool(name="cpsum", bufs=1, space="PSUM") as cps:
                gi_ps = [cps.tile([128, RD], f32, name=f"gi{m}")
                         for m in range(6)]
                for m in range(6):
                    ms = slice(m * 128, (m + 1) * 128)
                    nc.tensor.matmul(out=gi_ps[m][:], lhsT=wih_g[:, 0, ms],
                                     rhs=h_sb[0][:, DC], start=True, stop=False)
                    nc.tensor.matmul(out=gi_ps[m][:], lhsT=wih_g[:, 1, ms],
                                     rhs=h_sb[1][:, DC], start=False, stop=True)
                for k in range(2):
                    rg = wp.tile([128, RD], bf16, name=f"rg{k}")
                    zcg = wp.tile([128, RD], bf16, name=f"zcg{k}")
                    t1g = wp.tile([128, RD], f32, name=f"t1g{k}")
                    t2g = wp.tile([128, RD], f32, name=f"t2g{k}")
                    ng = wp.tile([128, RD], bf16, name=f"ng{k}")
                    nc.scalar.activation(out=rg[:], in_=gi_ps[k][:],
                                         func=AF.Sigmoid,
                                         bias=b_rz_g_pos[:, k:k + 1])
                    nc.scalar.activation(out=zcg[:], in_=gi_ps[2 + k][:],
                                         func=AF.Sigmoid, scale=-1.0,
                                         bias=b_rz_g_neg[:, k:k + 1])
                    nc.vector.tensor_scalar_mul(t1g[:], rg[:],
                                                b_hn_g[:, k:k + 1])
                    nc.vector.tensor_add(out=t2g[:], in0=t1g[:],
                                         in1=gi_ps[4 + k][:])
                    nc.scalar.activation(out=ng[:], in_=t2g[:], func=AF.Tanh,
                                         bias=b_in_g[:, k:k + 1])
                    nc.vector.tensor_mul(out=rnn[k][:], in0=zcg[:], in1=ng[:])

            # ---------------- 3 attention blocks ----------------
            for bk in range(3):
                with tc.tile_pool(name=f"apsum{bk}", bufs=1, space="PSUM") as ap:
                    u_sb = [wp.tile([128, RE], bf16, name=f"u{k}")
                            for k in range(2)]
                    for m in range(2):
                        ms = slice(m * 128, (m + 1) * 128)
                        u_ps = ap.tile([128, RE], f32, name=f"u_ps{m}")
                        t2_ps = ap.tile([128, RD], f32, name=f"t2_ps{m}")
                        nc.tensor.matmul(out=u_ps[:], lhsT=a1t[:, 0, ms],
                                         rhs=h_sb[0][:, EC], start=True, stop=False)
                        nc.tensor.matmul(out=u_ps[:], lhsT=a1t[:, 1, ms],
                                         rhs=h_sb[1][:, EC], start=False, stop=True)
                        nc.tensor.matmul(out=t2_ps[:], lhsT=a2t[:, 0, ms],
                                         rhs=rnn[0][:], start=True, stop=False)
                        nc.tensor.matmul(out=t2_ps[:], lhsT=a2t[:, 1, ms],
                                         rhs=rnn[1][:], start=False, stop=True)
                        t2_sb = wp.tile([128, RD], f32, name=f"t2sb{m}")
                        nc.scalar.copy(out=t2_sb[:], in_=t2_ps[:])
                        upre = wp.tile([128, RE], f32, name=f"upre{m}")
                        nc.vector.tensor_tensor(
                            out=upre[:].rearrange("p (b d) -> p b d", d=D),
                            in0=u_ps[:].rearrange("p (b d) -> p b d", d=D),
                            in1=t2_sb[:].unsqueeze(2).broadcast_to([128, RD, D]),
                            op=OP.add)
                        nc.scalar.activation(out=u_sb[m][:], in_=upre[:],
                                             func=AF.Tanh)

                    sc_ps = ap.tile([1, RE], f32, name="sc_ps")
                    nc.tensor.matmul(out=sc_ps[:], lhsT=attv[:, 0:1],
                                     rhs=u_sb[0][:], start=True, stop=False)
                    nc.tensor.matmul(out=sc_ps[:], lhsT=attv[:, 1:2],
                                     rhs=u_sb[1][:], start=False, stop=True)
                    e_sb = wp.tile([1, RE], f32, name="e_sb")
                    nc.scalar.activation(out=e_sb[:], in_=sc_ps[:], func=AF.Exp)
                    ssum = wp.tile([1, RD], f32, name="ssum")
                    nc.vector.tensor_reduce(
                        out=ssum[:], in_=e_sb[:].rearrange("p (b d) -> p b d", d=D),
                        axis=mybir.AxisListType.X, op=OP.add)
                    rs = wp.tile([1, RD], f32, name="rs")
                    nc.vector.reciprocal(out=rs[:], in_=ssum[:])
                    prob = wp.tile([1, RE], bf16, name="prob")
                    nc.vector.tensor_tensor(
                        out=prob[:].rearrange("p (b d) -> p b d", d=D),
                        in0=e_sb[:].rearrange("p (b d) -> p b d", d=D),
                        in1=rs[:].unsqueeze(2).broadcast_to([1, RD, D]),
                        op=OP.mult)
                    pbb_ps = ap.tile([128, RE], f32, name="pbb_ps")
                    nc.tensor.matmul(out=pbb_ps[:], lhsT=ones1[:], rhs=prob[:],
                                     start=True, stop=True)
                    rnn = [wp.tile([128, RD], bf16, name=f"rnnb{k}")
                           for k in range(2)]
                    for k in range(2):
                        ws = wp.tile([128, RE], f32, name=f"ws{k}")
                        nc.vector.tensor_mul(out=ws[:], in0=h_sb[k][:, EC],
                                             in1=pbb_ps[:])
                        red = wp.tile([128, RD], f32, name=f"red{k}")
                        nc.vector.tensor_reduce(
                            out=red[:],
                            in_=ws[:].rearrange("p (b d) -> p b d", d=D),
                            axis=mybir.AxisListType.X, op=OP.add)
                        nc.vector.tensor_copy(out=rnn[k][:], in_=red[:])

            # ---------------- FC head ----------------
            with tc.tile_pool(name="fpsum", bufs=1, space="PSUM") as fp:
                h1_sb = [wp.tile([128, RD], bf16, name=f"h1{m}")
                         for m in range(2)]
                for m in range(2):
                    ms = slice(m * 128, (m + 1) * 128)
                    h1_ps = fp.tile([128, RD], f32, name=f"h1ps{m}")
                    nc.tensor.matmul(out=h1_ps[:], lhsT=fc1t[:, 0, ms],
                                     rhs=rnn[0][:], start=True, stop=False)
                    nc.tensor.matmul(out=h1_ps[:], lhsT=fc1t[:, 1, ms],
                                     rhs=rnn[1][:], start=False, stop=True)
                    nc.scalar.activation(out=h1_sb[m][:], in_=h1_ps[:],
                                         func=AF.Relu, bias=b_fc1[:, m:m + 1])
                out_ps = fp.tile([1, RD], f32, name="out_ps")
                nc.tensor.matmul(out=out_ps[:], lhsT=fc2t[:, 0:1],
                                 rhs=h1_sb[0][:], start=True, stop=False)
                nc.tensor.matmul(out=out_ps[:], lhsT=fc2t[:, 1:2],
                                 rhs=h1_sb[1][:], start=False, stop=True)
                out_sb = wp.tile([1, RD], f32, name="out_sb")
                nc.scalar.activation(out=out_sb[:], in_=out_ps[:],
                                     func=AF.Identity, bias=b_fc2[0:1, 0:1])
                nc.sync.dma_start(out=y_d[:], in_=out_sb[:])

    _split_excess_waits(nc)
    return nc


def _lhsT3(w):
    """W [M, 256] -> lhsT tiles [128, 2, M]: [p, k, mc] = W[mc, k*128+p]."""
    wt = w.T.astype(np.float32)                       # [256, M]
    return np.ascontiguousarray(
        wt.reshape(2, 128, -1).transpose(1, 0, 2)).astype(NPBF)


def _wihb(w, bias):
    """W [M, 2], bias [M] -> zero-padded lhsT [128, M]: row0/1 = W cols, row2 = bias."""
    out = np.zeros((128, w.shape[0]), np.float32)
    out[0] = w[:, 0]
    out[1] = w[:, 1]
    out[2] = bias
    return out.astype(NPBF)


def _col2(v):
    """v [256] -> [128, 2] fp32 with column k = v[k*128:(k+1)*128]."""
    return np.ascontiguousarray(v.reshape(2, 128).T).astype(np.float32)


def _prep(inputs):
    el, ei = inputs["encoder_label"], inputs["encoder_input"]
    dl, di = inputs["decoder_label"], inputs["decoder_input"]
    Wih_e, Whh_e = inputs["Wih_e"], inputs["Whh_e"]
    bih_e, bhh_e = inputs["bih_e"], inputs["bhh_e"]
    Wih_d, Whh_d = inputs["Wih_d"], inputs["Whh_d"]
    bih_d, bhh_d = inputs["bih_d"], inputs["bhh_d"]
    Wih_g, bih_g, bhh_g = inputs["Wih_g"], inputs["bih_g"], inputs["bhh_g"]
    att_v, att_W = inputs["att_v"], inputs["att_W"]
    fc_W1, fc_b1 = inputs["fc_W1"], inputs["fc_b1"]
    fc_W2, fc_b2 = inputs["fc_W2"], inputs["fc_b2"]

    shared = {
        "whh_rz_e": _lhsT3(np.asarray(Whh_e)[:H2]),
        "whh_n_e": _lhsT3(np.asarray(Whh_e)[H2:]),
        "whh_rz_d": _lhsT3(np.asarray(Whh_d)[:H2]),
        "whh_n_d": _lhsT3(np.asarray(Whh_d)[H2:]),
        "wihb_rz_e": _wihb(np.asarray(Wih_e)[:H2],
                           (np.asarray(bih_e) + np.asarray(bhh_e))[:H2]),
        "wihb_n_e": _wihb(np.asarray(Wih_e)[H2:], np.asarray(bih_e)[H2:]),
        "wihb_rz_d": _wihb(np.asarray(Wih_d)[:H2],
                           (np.asarray(bih_d) + np.asarray(bhh_d))[:H2]),
        "wihb_n_d": _wihb(np.asarray(Wih_d)[H2:], np.asarray(bih_d)[H2:]),
        "b_hn_e": _col2(np.asarray(bhh_e)[H2:]),
        "b_hn_d": _col2(np.asarray(bhh_d)[H2:]),
        "wih_g": _lhsT3(np.asarray(Wih_g)),
        "b_rz_g_pos": _col2((np.asarray(bih_g) + np.asarray(bhh_g))[:H]),
        "b_rz_g_neg": _col2(-(np.asarray(bih_g) + np.asarray(bhh_g))[H:H2]),
        "b_hn_g": _col2(np.asarray(bhh_g)[H2:]),
        "b_in_g": _col2(np.asarray(bih_g)[H2:]),
        "a1t": _lhsT3(np.asarray(att_W)[:, :H]),
        "a2t": _lhsT3(np.asarray(att_W)[:, H:]),
        "attv": _col2(np.asarray(att_v)).astype(NPBF),
        "fc1t": _lhsT3(np.asarray(fc_W1)),
        "b_fc1": _col2(np.asarray(fc_b1)),
        "fc2t": _col2(np.asarray(fc_W2)[0]).astype(NPBF),
        "b_fc2": np.asarray(fc_b2).reshape(1, 1).astype(np.float32),
    }

    # per-core x tensors: [4, T, rows]; row r = b*D + d (b-major)
    lab_e = np.asarray(el).reshape(NCORES, BC, N, D).transpose(
        0, 2, 1, 3).reshape(NCORES, N, RE)
    dat_e = np.asarray(ei).reshape(NCORES, BC, N, D).transpose(
        0, 2, 1, 3).reshape(NCORES, N, RE)
    lab_d = np.asarray(dl).reshape(NCORES, BC, N).transpose(0, 2, 1)
    dat_d = np.asarray(di).reshape(NCORES, BC, N).transpose(0, 2, 1)

    in_maps = []
    for c in range(NCORES):
        xe = np.zeros((3, T, RE), np.float32)
        xe[0], xe[1], xe[2] = lab_e[c], dat_e[c], 1.0
        xd = np.zeros((3, T, RD), np.float32)
        xd[0], xd[1], xd[2] = lab_d[c], dat_d[c], 1.0
        m = dict(shared)
        m["xe"] = xe.astype(NPBF)
        m["xd"] = xd.astype(NPBF)
        in_maps.append(m)
    return in_maps


def kernel(**inputs):
    return _run(inputs, reps=1)


def _run(inputs, reps=1, mode="full"):
    key = (reps, mode)
    if key not in _BUILT:
        _BUILT[key] = _build(reps, mode)
    nc = _BUILT[key]
    in_maps = _prep(inputs)
    res = run_bass_kernel_spmd(nc, in_maps, core_ids=list(range(NCORES)),
                               trace=TRACE)
    LAST_RESULT[0] = res
    y = np.zeros((B, 1), np.float32)
    for c in range(NCORES):
        y[c * BC:(c + 1) * BC, 0] = res.results[c]["y"][0]
    return y



# revision 5
# speedup vs baseline: 1.3549x; 1.3549x over previous
"""Trainium2 Bass kernel for nn_Critic (RnnEncoder + attention critic).

Data-parallel over batch B=256 across 8 cores (32 batches/core).
Channel-transposed layout: GRU channels on partitions, batch rows on the
free dim.  Per core the 256 encoder rows (32 b x 8 d) + 32 decoder rows
are split into TWO software-pipelined column streams so the serial
per-step chain of one stream overlaps the other stream's work:

  stream A: enc rows of batches 0:16 (128 cols) + ALL 32 dec rows -> 160
  stream B: enc rows of batches 16:32 (128 cols)                  -> 128

GRU step restructure (vs. naive):
  * z-half of the rz weights is negated host-side, so one sigmoid gives
    zc = 1-z directly (PSUM holds -pre_z).
  * bhh_n enters the hn PSUM via a ones-row bias matmul, so the n-path
    is two plain DVE tensor_tensor ops: tmp = hn*r ; npre = tmp + inn.
  * h' = zc*n + (h - zc*h):  u = zc*h and v = h-u run OFF the critical
    chain on GpSimd; after tanh only w = zc*n and h' = w+v remain (DVE).
  * x-path matmuls are block-diagonal over K: shared rhs xc has enc
    inputs in rows 0..2 (label,data,1) and dec inputs in rows 3..5, so
    one matmul covers enc+dec columns with different weights/biases.
    All matmuls keep K=128 (zero-padded) to avoid K-regime switches.
"""

import numpy as np
import ml_dtypes

import concourse.bass as bass
import concourse.mybir as mybir
from concourse.tile import TileContext
from concourse.bass_utils import run_bass_kernel_spmd

B, N, D, H = 256, 128, 8, 256
NCORES = 8
BC = B // NCORES        # 32 batches per core
RE = BC * D             # 256 encoder rows per core
RD = BC                 # 32 decoder rows per core
T = N                   # 128 time steps
H2 = 2 * H              # 512 (r,z channels)

NS = 2                  # streams
SW = (160, 128)         # stream widths (cols): A = 128 enc + 32 dec, B = 128 enc
SBASE = (0, 160)        # xc column base per stream
SENC = (128, 128)       # encoder cols per stream
SDEC = (32, 0)          # decoder cols per stream

f32 = mybir.dt.float32
bf16 = mybir.dt.bfloat16
AF = mybir.ActivationFunctionType
OP = mybir.AluOpType
NPBF = ml_dtypes.bfloat16

TRACE = False
LAST_RESULT = [None]

_BUILT = {}


def _split_excess_waits(nc, max_waits=1):
    """This walrus build encodes at most one sync-wait per instruction for
    several instruction classes (CTRL/DMA).  Hoist extras onto nops."""
    for f in nc.m.functions:
        for bb in f.blocks:
            out = []
            for ins in bb.instructions:
                si = getattr(ins, "sync_info", None)
                if si is not None and len(si.on_wait) > max_waits:
                    waits = list(si.on_wait)
                    keep, extra = waits[-max_waits:], waits[:-max_waits]
                    for w in extra:
                        nop = mybir.InstNoOp(
                            name=nc.get_next_instruction_name(), ins=[], outs=[])
                        nop.engine = ins.engine
                        nop.sync_info = mybir.SyncInfo(on_wait=[w], on_update=[])
                        nc.register_instruction(nop, overwrite=True)
                        out.append(nop)
                    si.on_wait.clear()
                    for w in keep:
                        si.on_wait.append(w)
                out.append(ins)
            bb.instructions[:] = out


def _build():
    nc = bass.Bass(trn_type="TRN2")

    def din(name, shape, dt):
        return nc.dram_tensor(name, shape, dt, kind="ExternalInput")

    xc_d = din("xc", [6, T, RE + RD], bf16)
    whh_rz_e_d = din("whh_rz_e", [128, 2, H2], bf16)
    whh_n_e_d = din("whh_n_e", [128, 2, H], bf16)
    whh_rz_d_d = din("whh_rz_d", [128, 2, H2], bf16)
    whh_n_d_d = din("whh_n_d", [128, 2, H], bf16)
    wxrz_d = din("wxrz", [6, H2], bf16)
    wxn_d = din("wxn", [6, H], bf16)
    wbias_n_d = din("wbias_n", [6, H], bf16)
    wih_g_d = din("wih_g", [128, 2, 3 * H], bf16)
    b_rz_g_pos_d = din("b_rz_g_pos", [128, 2], f32)
    b_rz_g_neg_d = din("b_rz_g_neg", [128, 2], f32)
    b_hn_g_d = din("b_hn_g", [128, 2], f32)
    b_in_g_d = din("b_in_g", [128, 2], f32)
    a1t_d = din("a1t", [128, 2, H], bf16)
    a2t_d = din("a2t", [128, 2, H], bf16)
    attv_d = din("attv", [128, 2], bf16)
    fc1t_d = din("fc1t", [128, 2, H], bf16)
    b_fc1_d = din("b_fc1", [128, 2], f32)
    fc2t_d = din("fc2t", [128, 2], bf16)
    b_fc2_d = din("b_fc2", [1, 1], f32)

    y_d = nc.dram_tensor("y", [1, RD], f32, kind="ExternalOutput")

    def MS(m):
        return slice(m * 128, (m + 1) * 128)

    with TileContext(nc) as tc:
        with tc.tile_pool(name="const", bufs=1) as cp, \
             tc.tile_pool(name="state", bufs=1) as hp, \
             tc.tile_pool(name="work", bufs=2) as wp:

            def load(dram, shape, dt, name):
                t_ = cp.tile(shape, dt, name=name)
                nc.sync.dma_start(out=t_[:], in_=dram[:])
                return t_

            # x operand zero-padded to K=128: rows 0..2 enc (label,data,1),
            # rows 3..5 dec, rows 6..127 zero.
            xc = cp.tile([128, T, RE + RD], bf16, name="xc_s")
            nc.vector.memset(xc[:], 0.0)
            nc.sync.dma_start(out=xc[0:6, :, :], in_=xc_d[:])

            def loadpad6(dram, cols, name):
                t_ = cp.tile([128, cols], bf16, name=name)
                nc.vector.memset(t_[:], 0.0)
                nc.sync.dma_start(out=t_[0:6, :], in_=dram[:])
                return t_

            wxrz = loadpad6(wxrz_d, H2, "wxrz_s")
            wxn = loadpad6(wxn_d, H, "wxn_s")
            wbias_n = loadpad6(wbias_n_d, H, "wbiasn_s")

            whh_rz_e = load(whh_rz_e_d, [128, 2, H2], bf16, "whhrze_s")
            whh_n_e = load(whh_n_e_d, [128, 2, H], bf16, "whhne_s")
            whh_rz_d = load(whh_rz_d_d, [128, 2, H2], bf16, "whhrzd_s")
            whh_n_d = load(whh_n_d_d, [128, 2, H], bf16, "whhnd_s")
            wih_g = load(wih_g_d, [128, 2, 3 * H], bf16, "wihg_s")
            b_rz_g_pos = load(b_rz_g_pos_d, [128, 2], f32, "brzgp_s")
            b_rz_g_neg = load(b_rz_g_neg_d, [128, 2], f32, "brzgn_s")
            b_hn_g = load(b_hn_g_d, [128, 2], f32, "bhng_s")
            b_in_g = load(b_in_g_d, [128, 2], f32, "bing_s")
            a1t = load(a1t_d, [128, 2, H], bf16, "a1t_s")
            a2t = load(a2t_d, [128, 2, H], bf16, "a2t_s")
            attv = load(attv_d, [128, 2], bf16, "attv_s")
            fc1t = load(fc1t_d, [128, 2, H], bf16, "fc1t_s")
            b_fc1 = load(b_fc1_d, [128, 2], f32, "bfc1_s")
            fc2t = load(fc2t_d, [128, 2], bf16, "fc2t_s")
            b_fc2 = load(b_fc2_d, [1, 1], f32, "bfc2_s")

            # persistent hidden state per stream: [128, 2*W] bf16
            # (k-chunk c of h lives at cols c*W:(c+1)*W)
            h_cat = [hp.tile([128, 2 * SW[s]], bf16, name=f"hcat{s}")
                     for s in range(NS)]
            for s in range(NS):
                nc.vector.memset(h_cat[s][:], 0.0)

            ones1 = hp.tile([1, 128], bf16, name="ones1")
            nc.vector.memset(ones1[:], 1.0)

            def mm(o, w, r_, st, sp):
                nc.tensor.matmul(out=o, lhsT=w, rhs=r_, start=st, stop=sp)

            # ---------------- GRU time loop ----------------
            with tc.tile_pool(name="gpsum", bufs=1, space="PSUM") as gp:
                # per stream: r chunks at cols 0:W,W:2W of rps; negated-z
                # chunks in zps; hn and inn each their own bank
                rps = [gp.tile([128, 2 * SW[s]], f32, name=f"rps{s}")
                       for s in range(NS)]
                zps = [gp.tile([128, 2 * SW[s]], f32, name=f"zps{s}")
                       for s in range(NS)]
                hnps = [gp.tile([128, 2 * SW[s]], f32, name=f"hnps{s}")
                        for s in range(NS)]
                inps = [gp.tile([128, 2 * SW[s]], f32, name=f"inps{s}")
                        for s in range(NS)]

                for t in range(T):
                    for s in range(NS):
                        W = SW[s]
                        xs = xc[:, t, SBASE[s]:SBASE[s] + W]

                        def hslice(k, dec=False):
                            if dec:
                                return h_cat[s][:, k * W + 128:k * W + W]
                            return h_cat[s][:, k * W:k * W + 128]

                        # rz chunks: r0,r1 then zn0,zn1 (r first: sigmoid_r
                        # is on the critical chain)
                        for half, ps in ((0, rps[s]), (1, zps[s])):
                            for c in (0, 1):
                                chunk = half * 2 + c
                                po = c * W
                                out = ps[:, po:po + W]
                                oenc = ps[:, po:po + 128]
                                mm(out, wxrz[:, MS(chunk)], xs, True, False)
                                for k in (0, 1):
                                    mm(oenc, whh_rz_e[:, k, MS(chunk)],
                                       hslice(k), False,
                                       k == 1 and SDEC[s] == 0)
                                if SDEC[s]:
                                    odec = ps[:, po + 128:po + W]
                                    for k in (0, 1):
                                        mm(odec, whh_rz_d[:, k, MS(chunk)],
                                           hslice(k, True), False, k == 1)
                        # hn chunks (bias via ones-rows of xc)
                        for c in (0, 1):
                            out = hnps[s][:, c * W:c * W + W]
                            oenc = hnps[s][:, c * W:c * W + 128]
                            mm(out, wbias_n[:, MS(c)], xs, True, False)
                            for k in (0, 1):
                                mm(oenc, whh_n_e[:, k, MS(c)],
                                   hslice(k), False,
                                   k == 1 and SDEC[s] == 0)
                            if SDEC[s]:
                                odec = hnps[s][:, c * W + 128:c * W + W]
                                for k in (0, 1):
                                    mm(odec, whh_n_d[:, k, MS(c)],
                                       hslice(k, True), False, k == 1)
                        # inn chunks: single x matmul each
                        for c in (0, 1):
                            mm(inps[s][:, c * W:c * W + W],
                               wxn[:, MS(c)], xs, True, True)

                    # elementwise, stage-major across streams
                    rzsb = [wp.tile([128, 4 * SW[s]], bf16, name=f"rzsb{s}")
                            for s in range(NS)]
                    u_sb = [wp.tile([128, 2 * SW[s]], bf16, name=f"u{s}")
                            for s in range(NS)]
                    v_sb = [wp.tile([128, 2 * SW[s]], bf16, name=f"v{s}")
                            for s in range(NS)]
                    tmp_sb = [wp.tile([128, 2 * SW[s]], bf16, name=f"tmp{s}")
                              for s in range(NS)]
                    npre_sb = [wp.tile([128, 2 * SW[s]], bf16, name=f"npre{s}")
                               for s in range(NS)]
                    n_sb = [wp.tile([128, 2 * SW[s]], bf16, name=f"n{s}")
                            for s in range(NS)]
                    w_sb = [wp.tile([128, 2 * SW[s]], bf16, name=f"w{s}")
                            for s in range(NS)]

                    for s in range(NS):
                        W2 = 2 * SW[s]
                        nc.scalar.activation(
                            out=rzsb[s][:, 0:W2], in_=rps[s][:],
                            func=AF.Sigmoid)
                        nc.scalar.activation(
                            out=rzsb[s][:, W2:2 * W2], in_=zps[s][:],
                            func=AF.Sigmoid)
                    for s in range(NS):
                        W2 = 2 * SW[s]
                        # off-chain: u = zc*h ; v = h - u   (GpSimd)
                        nc.gpsimd.tensor_mul(out=u_sb[s][:],
                                             in0=rzsb[s][:, W2:2 * W2],
                                             in1=h_cat[s][:])
                        nc.gpsimd.tensor_sub(out=v_sb[s][:],
                                             in0=h_cat[s][:],
                                             in1=u_sb[s][:])
                    for s in range(NS):
                        W2 = 2 * SW[s]
                        # chain: tmp = hn*r ; npre = tmp+inn
                        nc.vector.tensor_mul(out=tmp_sb[s][:],
                                             in0=hnps[s][:],
                                             in1=rzsb[s][:, 0:W2])
                        nc.vector.tensor_add(out=npre_sb[s][:],
                                             in0=tmp_sb[s][:],
                                             in1=inps[s][:])
                    for s in range(NS):
                        nc.scalar.activation(out=n_sb[s][:], in_=npre_sb[s][:],
                                             func=AF.Tanh)
                    for s in range(NS):
                        W2 = 2 * SW[s]
                        # chain: w = zc*n ; h' = w + v
                        nc.vector.tensor_mul(out=w_sb[s][:],
                                             in0=rzsb[s][:, W2:2 * W2],
                                             in1=n_sb[s][:])
                        nc.vector.tensor_add(out=h_cat[s][:],
                                             in0=w_sb[s][:], in1=v_sb[s][:])

            # ---------------- critic GRU (single step, h0=0) ----------------
            # decoder hidden lives in stream A cols k*160+128 : k*160+160
            hd = [h_cat[0][:, k * 160 + 128:k * 160 + 160] for k in (0, 1)]
            rnn = [wp.tile([128, RD], bf16, name=f"rnn{k}") for k in range(2)]
            with tc.tile_pool(name="cpsum", bufs=1, space="PSUM") as cps:
                gi_ps = [cps.tile([128, RD], f32, name=f"gi{m}")
                         for m in range(6)]
                for m in range(6):
                    nc.tensor.matmul(out=gi_ps[m][:], lhsT=wih_g[:, 0, MS(m)],
                                     rhs=hd[0], start=True, stop=False)
                    nc.tensor.matmul(out=gi_ps[m][:], lhsT=wih_g[:, 1, MS(m)],
                                     rhs=hd[1], start=False, stop=True)
                for k in range(2):
                    rg = wp.tile([128, RD], bf16, name=f"rg{k}")
                    zcg = wp.tile([128, RD], bf16, name=f"zcg{k}")
                    t1g = wp.tile([128, RD], f32, name=f"t1g{k}")
                    t2g = wp.tile([128, RD], f32, name=f"t2g{k}")
                    ng = wp.tile([128, RD], bf16, name=f"ng{k}")
                    nc.scalar.activation(out=rg[:], in_=gi_ps[k][:],
                                         func=AF.Sigmoid,
                                         bias=b_rz_g_pos[:, k:k + 1])
                    nc.scalar.activation(out=zcg[:], in_=gi_ps[2 + k][:],
                                         func=AF.Sigmoid, scale=-1.0,
                                         bias=b_rz_g_neg[:, k:k + 1])
                    nc.vector.tensor_scalar_mul(t1g[:], rg[:],
                                                b_hn_g[:, k:k + 1])
                    nc.vector.tensor_add(out=t2g[:], in0=t1g[:],
                                         in1=gi_ps[4 + k][:])
                    nc.scalar.activation(out=ng[:], in_=t2g[:], func=AF.Tanh,
                                         bias=b_in_g[:, k:k + 1])
                    nc.vector.tensor_mul(out=rnn[k][:], in0=zcg[:], in1=ng[:])

            # ---------------- 3 attention blocks (per stream) ----------------
            # rnn holds all 32 batches; stream s covers batches s*16:(s+1)*16
            for bk in range(3):
                rnn_new = [wp.tile([128, RD], bf16, name=f"rnnb{bk}{k}")
                           for k in range(2)]
                for s in range(NS):
                    W = SW[s]
                    NB = 16              # batches in this stream
                    NE = SENC[s]         # 128 enc cols
                    ench = [h_cat[s][:, k * W:k * W + 128] for k in (0, 1)]
                    rn = [rnn[k][:, s * NB:(s + 1) * NB] for k in (0, 1)]
                    with tc.tile_pool(name=f"apsum{bk}{s}", bufs=1,
                                      space="PSUM") as ap:
                        u_at = [wp.tile([128, NE], bf16, name=f"uat{k}")
                                for k in range(2)]
                        for m in range(2):
                            u_ps = ap.tile([128, NE], f32, name=f"u_ps{m}")
                            t2_ps = ap.tile([128, NB], f32, name=f"t2_ps{m}")
                            nc.tensor.matmul(out=u_ps[:], lhsT=a1t[:, 0, MS(m)],
                                             rhs=ench[0], start=True, stop=False)
                            nc.tensor.matmul(out=u_ps[:], lhsT=a1t[:, 1, MS(m)],
                                             rhs=ench[1], start=False, stop=True)
                            nc.tensor.matmul(out=t2_ps[:], lhsT=a2t[:, 0, MS(m)],
                                             rhs=rn[0], start=True, stop=False)
                            nc.tensor.matmul(out=t2_ps[:], lhsT=a2t[:, 1, MS(m)],
                                             rhs=rn[1], start=False, stop=True)
                            t2_sb = wp.tile([128, NB], f32, name=f"t2sb{m}")
                            nc.scalar.copy(out=t2_sb[:], in_=t2_ps[:])
                            upre = wp.tile([128, NE], f32, name=f"upre{m}")
                            nc.vector.tensor_tensor(
                                out=upre[:].rearrange("p (b d) -> p b d", d=D),
                                in0=u_ps[:].rearrange("p (b d) -> p b d", d=D),
                                in1=t2_sb[:].unsqueeze(2).broadcast_to(
                                    [128, NB, D]),
                                op=OP.add)
                            nc.scalar.activation(out=u_at[m][:], in_=upre[:],
                                                 func=AF.Tanh)

                        sc_ps = ap.tile([1, NE], f32, name="sc_ps")
                        nc.tensor.matmul(out=sc_ps[:], lhsT=attv[:, 0:1],
                                         rhs=u_at[0][:], start=True, stop=False)
                        nc.tensor.matmul(out=sc_ps[:], lhsT=attv[:, 1:2],
                                         rhs=u_at[1][:], start=False, stop=True)
                        e_sb = wp.tile([1, NE], f32, name="e_sb")
                        nc.scalar.activation(out=e_sb[:], in_=sc_ps[:],
                                             func=AF.Exp)
                        ssum = wp.tile([1, NB], f32, name="ssum")
                        nc.vector.tensor_reduce(
                            out=ssum[:],
                            in_=e_sb[:].rearrange("p (b d) -> p b d", d=D),
                            axis=mybir.AxisListType.X, op=OP.add)
                        rs = wp.tile([1, NB], f32, name="rs")
                        nc.vector.reciprocal(out=rs[:], in_=ssum[:])
                        prob = wp.tile([1, NE], bf16, name="prob")
                        nc.vector.tensor_tensor(
                            out=prob[:].rearrange("p (b d) -> p b d", d=D),
                            in0=e_sb[:].rearrange("p (b d) -> p b d", d=D),
                            in1=rs[:].unsqueeze(2).broadcast_to([1, NB, D]),
                            op=OP.mult)
                        pbb_ps = ap.tile([128, NE], f32, name="pbb_ps")
                        nc.tensor.matmul(out=pbb_ps[:], lhsT=ones1[:],
                                         rhs=prob[:], start=True, stop=True)
                        for k in range(2):
                            ws = wp.tile([128, NE], f32, name=f"ws{k}")
                            nc.vector.tensor_mul(out=ws[:], in0=ench[k],
                                                 in1=pbb_ps[:])
                            red = wp.tile([128, NB], f32, name=f"red{k}")
                            nc.vector.tensor_reduce(
                                out=red[:],
                                in_=ws[:].rearrange("p (b d) -> p b d", d=D),
                                axis=mybir.AxisListType.X, op=OP.add)
                            nc.vector.tensor_copy(
                                out=rnn_new[k][:, s * NB:(s + 1) * NB],
                                in_=red[:])
                rnn = rnn_new

            # ---------------- FC head ----------------
            with tc.tile_pool(name="fpsum", bufs=1, space="PSUM") as fp:
                h1_sb = [wp.tile([128, RD], bf16, name=f"h1{m}")
                         for m in range(2)]
                for m in range(2):
                    h1_ps = fp.tile([128, RD], f32, name=f"h1ps{m}")
                    nc.tensor.matmul(out=h1_ps[:], lhsT=fc1t[:, 0, MS(m)],
                                     rhs=rnn[0][:], start=True, stop=False)
                    nc.tensor.matmul(out=h1_ps[:], lhsT=fc1t[:, 1, MS(m)],
                                     rhs=rnn[1][:], start=False, stop=True)
                    nc.scalar.activation(out=h1_sb[m][:], in_=h1_ps[:],
                                         func=AF.Relu, bias=b_fc1[:, m:m + 1])
                out_ps = fp.tile([1, RD], f32, name="out_ps")
                nc.tensor.matmul(out=out_ps[:], lhsT=fc2t[:, 0:1],
                                 rhs=h1_sb[0][:], start=True, stop=False)
                nc.tensor.matmul(out=out_ps[:], lhsT=fc2t[:, 1:2],
                                 rhs=h1_sb[1][:], start=False, stop=True)
                out_sb = wp.tile([1, RD], f32, name="out_sb")
                nc.scalar.activation(out=out_sb[:], in_=out_ps[:],
                                     func=AF.Identity, bias=b_fc2[0:1, 0:1])
                nc.sync.dma_start(out=y_d[:], in_=out_sb[:])

    _split_excess_waits(nc)
    return nc


def _lhsT3(w):
    """W [M, 256] -> lhsT tiles [128, 2, M]: [p, k, mc] = W[mc, k*128+p]."""
    wt = w.T.astype(np.float32)                       # [256, M]
    return np.ascontiguousarray(
        wt.reshape(2, 128, -1).transpose(1, 0, 2)).astype(NPBF)


def _col2(v):
    """v [256] -> [128, 2] fp32 with column k = v[k*128:(k+1)*128]."""
    return np.ascontiguousarray(v.reshape(2, 128).T).astype(np.float32)


def _negz(a):
    """Negate the z-half (rows H:2H) of an rz-block array [2H, ...]."""
    out = np.array(a, dtype=np.float32)
    out[H:H2] = -out[H:H2]
    return out


def _prep(inputs):
    el, ei = inputs["encoder_label"], inputs["encoder_input"]
    dl, di = inputs["decoder_label"], inputs["decoder_input"]
    Wih_e, Whh_e = np.asarray(inputs["Wih_e"]), np.asarray(inputs["Whh_e"])
    bih_e, bhh_e = np.asarray(inputs["bih_e"]), np.asarray(inputs["bhh_e"])
    Wih_d, Whh_d = np.asarray(inputs["Wih_d"]), np.asarray(inputs["Whh_d"])
    bih_d, bhh_d = np.asarray(inputs["bih_d"]), np.asarray(inputs["bhh_d"])
    Wih_g, bih_g, bhh_g = (np.asarray(inputs["Wih_g"]),
                           np.asarray(inputs["bih_g"]),
                           np.asarray(inputs["bhh_g"]))
    att_v, att_W = np.asarray(inputs["att_v"]), np.asarray(inputs["att_W"])
    fc_W1, fc_b1 = np.asarray(inputs["fc_W1"]), np.asarray(inputs["fc_b1"])
    fc_W2, fc_b2 = np.asarray(inputs["fc_W2"]), np.asarray(inputs["fc_b2"])

    # x/bias weight rows (K-block-diagonal): rows 0..2 enc, 3..5 dec
    wxrz = np.zeros((6, H2), np.float32)
    wxrz[0] = _negz(Wih_e[:H2, 0])
    wxrz[1] = _negz(Wih_e[:H2, 1])
    wxrz[2] = _negz(bih_e[:H2] + bhh_e[:H2])
    wxrz[3] = _negz(Wih_d[:H2, 0])
    wxrz[4] = _negz(Wih_d[:H2, 1])
    wxrz[5] = _negz(bih_d[:H2] + bhh_d[:H2])
    wxn = np.zeros((6, H), np.float32)
    wxn[0], wxn[1], wxn[2] = Wih_e[H2:, 0], Wih_e[H2:, 1], bih_e[H2:]
    wxn[3], wxn[4], wxn[5] = Wih_d[H2:, 0], Wih_d[H2:, 1], bih_d[H2:]
    wbias_n = np.zeros((6, H), np.float32)
    wbias_n[2] = bhh_e[H2:]
    wbias_n[5] = bhh_d[H2:]

    shared = {
        "whh_rz_e": _lhsT3(_negz(Whh_e[:H2])),
        "whh_n_e": _lhsT3(Whh_e[H2:]),
        "whh_rz_d": _lhsT3(_negz(Whh_d[:H2])),
        "whh_n_d": _lhsT3(Whh_d[H2:]),
        "wxrz": wxrz.astype(NPBF),
        "wxn": wxn.astype(NPBF),
        "wbias_n": wbias_n.astype(NPBF),
        "wih_g": _lhsT3(Wih_g),
        "b_rz_g_pos": _col2((bih_g + bhh_g)[:H]),
        "b_rz_g_neg": _col2(-(bih_g + bhh_g)[H:H2]),
        "b_hn_g": _col2(bhh_g[H2:]),
        "b_in_g": _col2(bih_g[H2:]),
        "a1t": _lhsT3(att_W[:, :H]),
        "a2t": _lhsT3(att_W[:, H:]),
        "attv": _col2(att_v).astype(NPBF),
        "fc1t": _lhsT3(fc_W1),
        "b_fc1": _col2(fc_b1),
        "fc2t": _col2(fc_W2[0]).astype(NPBF),
        "b_fc2": fc_b2.reshape(1, 1).astype(np.float32),
    }

    # per-core x tensor [6, T, 288]:
    #   cols 0:128   enc batches 0:16  (b-major, row = b*D+d)  <- stream A
    #   cols 128:160 dec batches 0:32                          <- stream A
    #   cols 160:288 enc batches 16:32                         <- stream B
    lab_e = np.asarray(el).reshape(NCORES, BC, N, D).transpose(
        0, 2, 1, 3).reshape(NCORES, N, RE)
    dat_e = np.asarray(ei).reshape(NCORES, BC, N, D).transpose(
        0, 2, 1, 3).reshape(NCORES, N, RE)
    lab_d = np.asarray(dl).reshape(NCORES, BC, N).transpose(0, 2, 1)
    dat_d = np.asarray(di).reshape(NCORES, BC, N).transpose(0, 2, 1)

    in_maps = []
    for c in range(NCORES):
        xcm = np.zeros((6, T, RE + RD), np.float32)
        xcm[0, :, 0:128] = lab_e[c][:, 0:128]
        xcm[1, :, 0:128] = dat_e[c][:, 0:128]
        xcm[2, :, 0:128] = 1.0
        xcm[3, :, 128:160] = lab_d[c]
        xcm[4, :, 128:160] = dat_d[c]
        xcm[5, :, 128:160] = 1.0
        xcm[0, :, 160:288] = lab_e[c][:, 128:256]
        xcm[1, :, 160:288] = dat_e[c][:, 128:256]
        xcm[2, :, 160:288] = 1.0
        m = dict(shared)
        m["xc"] = xcm.astype(NPBF)
        in_maps.append(m)
    return in_maps


def kernel(**inputs):
    if "k" not in _BUILT:
        _BUILT["k"] = _build()
    nc = _BUILT["k"]
    in_maps = _prep(inputs)
    res = run_bass_kernel_spmd(nc, in_maps, core_ids=list(range(NCORES)),
                               trace=TRACE)
    LAST_RESULT[0] = res
    y = np.zeros((B, 1), np.float32)
    for c in range(NCORES):
        y[c * BC:(c + 1) * BC, 0] = res.results[c]["y"][0]
    return y
